# revision 1
# baseline (speedup 1.0000x reference)
"""Trainium2 Bass kernel for nn_DepthWiseSepConv (depthwise 5x5 + BN+hardswish
+ pointwise 1x1 + squeeze-excite gating + BN), data-parallel over batch on
8 NeuronCores.

Self-contained: hardcodes all shapes from the problem spec.

Per-core layout strategy (B_loc = 8 images per core):
  - Depthwise conv: partitions = (4 channels x 28 rows of H). For each of the
    5 kernel columns dx, one matmul with a host-built block-diagonal Toeplitz
    matrix (contracting h_in -> h_out) against x shifted by dx along W (zero
    padded in SBUF). The 5 matmuls accumulate in PSUM.
  - BN1 + hardswish fused: ACT relu(psum*s1 + t1+3), then
    act = (a-3) * min(a/6, 1).
  - Two TensorE transpose stages to reach channel-major [c, (b,h,w)] layout
    for the pointwise conv.
  - SE: DVE free-dim reduce for the mean, two small matmuls, hardswish.
  - Pointwise conv: [120c x 120o] matmul tiles, N=392 (half an image),
    epilogue fuses +pw_b, *g (SE gate), BN2.
"""

import sys

sys.path.insert(0, "/opt/trn_rl_repo")

import numpy as np
import ml_dtypes

import concourse.bass as bass
import concourse.mybir as mybir
import concourse.tile as tile
from concourse import bacc
from concourse.bass_utils import run_bass_kernel_spmd
from concourse.masks import make_identity

# ---------------------------------------------------------------- constants
N_CORES = 8
B, C, H, W = 64, 240, 28, 28
NB = B // N_CORES          # images per core
KK = 5                      # depthwise kernel size
G = C // 4                  # channel groups of 4 -> 60
R = 60                      # SE reduction dim
Cout = 240
HW = H * W                  # 784
EPS = 1e-5
WP = 36                     # padded W in SBUF x tiles (w in [-2, 34))
PIX = NB * HW               # 6272 pixels per core

CFG = {
    # dtype of DW + PW matmul operands: "float32" | "float32r" | "bfloat16"
    "mm_dt": "float32r",
    # dtype of activation storage / transposes: "float32" | "bfloat16"
    "act_dt": "float32",
    # DW psum free width: 28 (exact) or 32 (padded, helps float32r)
    "wout": 32,
    # pack DW Toeplitz as 4x[32,32] tile_position blocks (3x less weight DMA)
    "packed": False,
    # DRAM storage dtype of toep; float16 halves DMA, cast to f32 in flight
    "toep_store": "float16",
    # transfer only the 4 diagonal 28x28 blocks of each Toeplitz (4x less
    # DMA); persistent pre-zeroed SBUF buffers, manual 3-way rotation
    "toep_compact": False,
    # debug: emit only a prefix of the phases ("a"|"ab"|"abc"|"")
    "stop_after": "",
    # x DMA batching: groups loaded per DMA (1 or 2)
    "xbatch": 1,
    # rotation depth for x/toep persistent buffers
    "nrot": 4,
    # DW psum pool depth
    "dwbufs": 4,
}

_DT = {
    "float32": mybir.dt.float32,
    "float32r": mybir.dt.float32r,
    "bfloat16": mybir.dt.bfloat16,
}
_NPDT = {
    "float32": np.float32,
    "float32r": np.float32,
    "bfloat16": ml_dtypes.bfloat16,
}


def _f32v(ap):
    """View a float32r AP as plain float32 (for non-matmul readers)."""
    if ap.dtype == mybir.dt.float32r:
        return ap.bitcast(mybir.dt.float32)
    return ap


# ---------------------------------------------------------------- builder
_BUILD_CACHE = {}


def build_nc(cfg_key=None):
    cfg = dict(CFG)
    if cfg_key is not None:
        cfg.update(cfg_key)
    key = tuple(sorted(cfg.items()))
    if key in _BUILD_CACHE:
        return _BUILD_CACHE[key]

    mm_dt = _DT[cfg["mm_dt"]]
    act_dt = _DT[cfg["act_dt"]]
    WOUT = cfg["wout"]
    dw_r = cfg["mm_dt"] == "float32r"
    pw_r = dw_r and cfg["act_dt"] == "float32"
    # dtype of the PW matmul operands (weights + transposed activations)
    pw_dt = mybir.dt.float32r if pw_r else act_dt

    nc = bacc.Bacc("TRN2", target_bir_lowering=False, debug=False,
                   num_devices=N_CORES)

    packed = cfg["packed"]
    DWP = 128 if packed else 112     # DW partition count
    HB = 32 if packed else H         # per-channel partition block
    toep_st = mm_dt
    if cfg["toep_store"] == "float16" and cfg["mm_dt"] != "bfloat16":
        toep_st = mybir.dt.float16

    f32 = mybir.dt.float32
    x_dram_dt = f32 if cfg["mm_dt"] == "bfloat16" else mm_dt
    x_p = nc.declare_dram_parameter("x", [NB, C, H, W], x_dram_dt,
                                    isOutput=False)
    if packed:
        toep_p = nc.declare_dram_parameter("toep", [G, 4, 32, KK, 32], toep_st,
                                           isOutput=False)
    elif cfg["toep_compact"]:
        toep_p = nc.declare_dram_parameter("toep", [G, 4, H, KK, H], toep_st,
                                           isOutput=False)
    else:
        toep_p = nc.declare_dram_parameter("toep", [G, 112, KK, 112], toep_st,
                                           isOutput=False)
    bn1s_p = nc.declare_dram_parameter("bn1s", [DWP, G], f32, isOutput=False)
    bn1b_p = nc.declare_dram_parameter("bn1b", [DWP, G], f32, isOutput=False)
    pwl_p = nc.declare_dram_parameter("pwl", [2, 120, 2, 120], pw_dt,
                                      isOutput=False)
    se1l_p = nc.declare_dram_parameter("se1l", [2, 120, R], f32, isOutput=False)
    se1b_p = nc.declare_dram_parameter("se1b", [R, 1], f32, isOutput=False)
    se2l_p = nc.declare_dram_parameter("se2l", [R, 2, 120], f32, isOutput=False)
    se2b3_p = nc.declare_dram_parameter("se2b3", [120, 2], f32, isOutput=False)
    bn2s_p = nc.declare_dram_parameter("bn2s", [120, 2], f32, isOutput=False)
    bn2sb_p = nc.declare_dram_parameter("bn2sb", [120, 2], f32, isOutput=False)
    bn2t_p = nc.declare_dram_parameter("bn2t", [120, 2], f32, isOutput=False)
    # zero-fill source (walrus rejects Memset on float32r tiles)
    zeros_p = nc.declare_dram_parameter("zeros", [128, 640], mm_dt,
                                        isOutput=False)
    y_p = nc.declare_dram_parameter("y", [NB, Cout, H, W], f32, isOutput=True)

    AL = mybir.AluOpType

    with tile.TileContext(nc) as tc:
        cst = tc.alloc_tile_pool(name="cst", bufs=1)
        pers = tc.alloc_tile_pool(name="pers", bufs=1)

        # ---- constants in SBUF
        bn1s_sb = cst.tile([DWP, G], f32)
        nc.sync.dma_start(bn1s_sb[:], bn1s_p[:])
        bn1b_sb = cst.tile([DWP, G], f32)
        nc.sync.dma_start(bn1b_sb[:], bn1b_p[:])
        pwl_sb = cst.tile([120, 2, 2, 120], pw_dt)  # [K=c, kc, mo, M=o]
        nc.sync.dma_start(pwl_sb[:], pwl_p[:].rearrange("kc k mo m -> k kc mo m"))
        se1l_sb = cst.tile([120, 2, R], f32)
        nc.sync.dma_start(se1l_sb[:], se1l_p[:].rearrange("kc k r -> k kc r"))
        se1b_sb = cst.tile([R, 1], f32)
        nc.sync.dma_start(se1b_sb[:], se1b_p[:])
        se2l_sb = cst.tile([R, 2, 120], f32)
        nc.sync.dma_start(se2l_sb[:], se2l_p[:])
        se2b3_sb = cst.tile([120, 2], f32)
        nc.sync.dma_start(se2b3_sb[:], se2b3_p[:])
        bn2s_sb = cst.tile([120, 2], f32)
        nc.sync.dma_start(bn2s_sb[:], bn2s_p[:])
        bn2sb_sb = cst.tile([120, 2], f32)
        nc.sync.dma_start(bn2sb_sb[:], bn2sb_p[:])
        bn2t_sb = cst.tile([120, 2], f32)
        nc.sync.dma_start(bn2t_sb[:], bn2t_p[:])

        ident = cst.tile([128, 128], act_dt)
        make_identity(nc, ident[:])

        # persistent activation buffers
        # ActT[ch]: [(b4,w28)=112, q, (g_local, c4, h) = 30*112]
        ActT = [pers.tile([112, 2, 30 * 112], act_dt, name=f"actt_{ch}")
                for ch in range(2)]
        # PWrhs[ch]: [c=120, (b, h, w) = 6272]
        PWrhs = [pers.tile([120, PIX], pw_dt, name=f"pwrhs_{ch}")
                 for ch in range(2)]
        g_sb = [pers.tile([120, NB], f32, name=f"gate_{mo}") for mo in range(2)]

        # x rearranged for DW rhs: dims (c4, h, g, b, w)
        x_r = x_p[:].rearrange("b (g c) h w -> c h g b w", c=4)
        # merged (c h) partition form for the unpacked single-DMA load
        x_rm = x_p[:].rearrange("b (g c) h w -> (c h) g b w", c=4)

        # persistent DW input buffers, manual 3-way rotation: zero padding is
        # written once, per-group DMAs only overwrite the payload regions
        NROT = cfg["nrot"]
        XB = cfg["xbatch"]
        x_bufs = [pers.tile([DWP, XB, NB, WP], mm_dt, name=f"x_rot{i}")
                  for i in range(NROT)]
        zx = zeros_p[:, :XB * NB * WP].rearrange(
            "p (xb nb wp) -> p xb nb wp", xb=XB, nb=NB)
        for xb in x_bufs:
            nc.sync.dma_start(xb[:], zx[:DWP])
        toep_bufs = None
        if cfg["toep_compact"] and not packed:
            toep_bufs = [pers.tile([112, KK, 112], mm_dt, name=f"tp_rot{i}")
                         for i in range(NROT)]
            zt = zeros_p[:, :KK * 112].rearrange("p (k m) -> p k m", k=KK)
            for tb in toep_bufs:
                nc.sync.dma_start(tb[:], zt[:112])

        # ================= Phase A: depthwise + BN1 + hardswish + T1
        with tc.tile_pool(name="pa", bufs=3) as pa, \
             tc.tile_pool(name="pa2", bufs=4) as pa2, \
             tc.tile_pool(name="dwps", bufs=cfg["dwbufs"], space="PSUM") as dwps, \
             tc.tile_pool(name="t1ps", bufs=3, space="PSUM") as t1ps:
            for g in range(G):
                gb, gi = divmod(g, XB)
                x_gb = x_bufs[gb % NROT]
                x_g = x_gb[:, gi]
                x_dma = (nc.gpsimd.dma_start
                         if cfg["mm_dt"] == "bfloat16" else nc.sync.dma_start)
                if gi == 0:
                    if packed:
                        # SBUF APs have a single partition dim -> one DMA
                        # per 32-row channel block
                        for ci in range(4):
                            x_dma(x_gb[32 * ci:32 * ci + H, 0, :, 2:2 + W],
                                  x_r[ci, :, g])
                    elif XB == 1:
                        x_dma(x_gb[:, 0, :, 2:2 + W], x_rm[:, g])
                    else:
                        x_dma(x_gb[:, :, :, 2:2 + W],
                              x_rm[:, g:g + XB])
                toep_dma = (nc.gpsimd.dma_start if toep_st != mm_dt
                            else nc.sync.dma_start)
                if packed:
                    toep_g = pa.tile([128, KK, 32], mm_dt, tag="toep_g")
                    toep_dma(toep_g[:],
                             toep_p[g].rearrange("c e dx m -> (c e) dx m"))
                elif cfg["toep_compact"]:
                    toep_g = toep_bufs[g % NROT]
                    for ci in range(4):
                        toep_dma(
                            toep_g[H * ci:H * ci + H, :, H * ci:H * ci + H],
                            toep_p[g, ci])
                else:
                    toep_g = pa.tile([112, KK, 112], mm_dt, tag="toep_g")
                    toep_dma(toep_g[:], toep_p[g])

                ps = dwps.tile([DWP, NB, WOUT], f32, tag="dw")
                for dx in range(KK):
                    if packed:
                        # one accumulation group for the whole psum region:
                        # start clears has_written bank-wide; per-element
                        # has_written handles first-write-overwrite for the
                        # other 32-row blocks
                        for ci in range(4):
                            nc.tensor.matmul(
                                ps[32 * ci:32 * ci + 32],
                                toep_g[32 * ci:32 * ci + 32, dx, :],
                                x_g[32 * ci:32 * ci + 32, :, dx:dx + WOUT],
                                start=(dx == 0 and ci == 0),
                                stop=(dx == KK - 1 and ci == 3),
                                tile_position=(32 * ci, 32 * ci),
                            )
                    else:
                        nc.tensor.matmul(
                            ps[:],
                            toep_g[:, dx, :],
                            x_g[:, :, dx:dx + WOUT],
                            start=(dx == 0),
                            stop=(dx == KK - 1),
                        )

                a_g = pa2.tile([DWP, NB, WOUT], f32, tag="a_g")
                nc.scalar.activation(a_g[:], ps[:],
                                     mybir.ActivationFunctionType.Relu,
                                     bias=bn1b_sb[:, g:g + 1],
                                     scale=bn1s_sb[:, g:g + 1])
                a_v = a_g[:, :, 0:W]
                m_g = pa2.tile([DWP, NB, W], f32, tag="m_g")
                nc.gpsimd.tensor_scalar(m_g[:], a_v, 1.0 / 6.0, 1.0,
                                        AL.mult, AL.min)
                act_g = pa2.tile([DWP, NB, W], act_dt, tag="act_g")
                nc.vector.scalar_tensor_tensor(act_g[:], a_v, 3.0, m_g[:],
                                               AL.subtract, AL.mult)

                ch, gl = (0, g) if g < 30 else (1, g - 30)
                tp = t1ps.tile([112, 2, DWP], act_dt, tag="t1")
                for q in range(2):
                    nc.tensor.transpose(tp[:, q, :],
                                        act_g[:, 4 * q:4 * q + 4, :],
                                        ident[:DWP, :DWP])
                # select real (c4, h) columns out of each DWP block
                tp_sel = tp[:].rearrange("p q (c e) -> p q c e", c=4)[
                    :, :, :, 0:H]
                nc.scalar.copy(
                    ActT[ch][:, :, gl * 112:(gl + 1) * 112].rearrange(
                        "p q (c e) -> p q c e", c=4),
                    tp_sel)

        # ================= Phase B: T2 -> channel-major PWrhs
        with tc.tile_pool(name="t2ps", bufs=6, space="PSUM") as t2ps:
            for ch in range(2):
                for q in range(2):
                    src4 = ActT[ch][:].rearrange(
                        "p q (gl c e) -> p q gl c e", gl=30, c=4)
                    dst4 = PWrhs[ch][:].rearrange(
                        "p (b hh w) -> p b hh w", b=NB, hh=H)
                    for h0 in range(0, H, 4):
                        tp = t2ps.tile([120, 4, 112], act_dt, tag="t2")
                        for hi in range(4):
                            # 120 cols: (g_local str 112) x (c4 str 28), off h
                            nc.tensor.transpose(tp[:, hi, :],
                                                src4[:, q, :, :, h0 + hi],
                                                ident[:112, :112])
                        nc.vector.tensor_copy(
                            dst4[:, 4 * q:4 * q + 4, h0:h0 + 4, :],
                            tp[:].rearrange("p hh (b w) -> p b hh w", b=4))

        # ================= Phase C: squeeze-excite
        with tc.tile_pool(name="se", bufs=1) as sep, \
             tc.tile_pool(name="seps", bufs=2, space="PSUM") as seps:
            s_sb = [sep.tile([120, NB], f32, name=f"s_{ch}") for ch in range(2)]
            for ch in range(2):
                nc.vector.tensor_reduce(
                    s_sb[ch][:],
                    _f32v(PWrhs[ch][:]).rearrange("p (b f) -> p b f", b=NB),
                    mybir.AxisListType.X, AL.add)
            ps1 = seps.tile([R, NB], f32, tag="se1")
            for ch in range(2):
                nc.tensor.matmul(ps1[:], se1l_sb[:, ch, :], s_sb[ch][:],
                                 start=(ch == 0), stop=(ch == 1))
            h1 = sep.tile([R, NB], f32)
            nc.scalar.activation(h1[:], ps1[:],
                                 mybir.ActivationFunctionType.Relu,
                                 bias=se1b_sb[:, 0:1])
            for mo in range(2):
                ps2 = seps.tile([120, NB], f32, tag="se2")
                nc.tensor.matmul(ps2[:], se2l_sb[:, mo, :], h1[:],
                                 start=True, stop=True)
                a2 = sep.tile([120, NB], f32, name=f"a2_{mo}")
                nc.scalar.activation(a2[:], ps2[:],
                                     mybir.ActivationFunctionType.Relu,
                                     bias=se2b3_sb[:, mo:mo + 1])
                m2 = sep.tile([120, NB], f32, name=f"m2_{mo}")
                nc.vector.tensor_scalar(m2[:], a2[:], 1.0 / 6.0, 1.0,
                                        AL.mult, AL.min)
                nc.vector.scalar_tensor_tensor(g_sb[mo][:], a2[:], 3.0, m2[:],
                                               AL.subtract, AL.mult)

        # ================= Phase D: pointwise conv + gate + BN2 + output
        NT = 392  # half an image
        with tc.tile_pool(name="pd", bufs=6) as pd, \
             tc.tile_pool(name="pdps", bufs=4, space="PSUM") as pdps:
            for mo in range(2):
                for b in range(NB):
                    for nt in range(2):
                        off = b * HW + nt * NT
                        ps = pdps.tile([120, NT], f32, tag="pw")
                        for kc in range(2):
                            nc.tensor.matmul(
                                ps[:],
                                pwl_sb[:, kc, mo, :],
                                PWrhs[kc][:, off:off + NT],
                                start=(kc == 0), stop=(kc == 1))
                        e2 = pd.tile([120, NT], f32, tag="e2")
                        nc.scalar.activation(
                            e2[:], ps[:],
                            mybir.ActivationFunctionType.Identity,
                            bias=bn2sb_sb[:, mo:mo + 1],
                            scale=bn2s_sb[:, mo:mo + 1])
                        f_t = pd.tile([120, NT], f32, tag="f_t")
                        nc.vector.tensor_tensor(
                            f_t[:], e2[:],
                            g_sb[mo][:, b:b + 1].to_broadcast((120, NT)),
                            AL.mult)
                        o_t = pd.tile([120, NT], f32, tag="o_t")
                        nc.gpsimd.tensor_scalar(o_t[:], f_t[:],
                                                bn2t_sb[:, mo:mo + 1], None,
                                                AL.add)
                        y_ap = y_p[b, mo * 120:(mo + 1) * 120].rearrange(
                            "c h w -> c (h w)")[:, nt * NT:(nt + 1) * NT]
                        nc.sync.dma_start(y_ap, o_t[:])

        pers.release()
        cst.release()

    nc.compile()
    _BUILD_CACHE[key] = nc
    return nc


# ---------------------------------------------------------------- host prep
def prep_inputs(inputs, cfg_key=None):
    cfg = dict(CFG)
    if cfg_key is not None:
        cfg.update(cfg_key)
    mmnp = _NPDT[cfg["mm_dt"]]
    f32 = np.float32

    x = np.asarray(inputs["x"], f32)
    dw_w = np.asarray(inputs["dw_w"], f32)      # [C,1,5,5]
    dw_b = np.asarray(inputs["dw_b"], f32)
    bn1_g = np.asarray(inputs["bn1_g"], f32)
    bn1_b = np.asarray(inputs["bn1_b"], f32)
    bn1_m = np.asarray(inputs["bn1_m"], f32)
    bn1_v = np.asarray(inputs["bn1_v"], f32)
    pw_w = np.asarray(inputs["pw_w"], f32)      # [Cout, C]
    pw_b = np.asarray(inputs["pw_b"], f32)
    se_w1 = np.asarray(inputs["se_w1"], f32)    # [R, C]
    se_b1 = np.asarray(inputs["se_b1"], f32)
    se_w2 = np.asarray(inputs["se_w2"], f32)    # [Cout, R]
    se_b2 = np.asarray(inputs["se_b2"], f32)
    bn2_g = np.asarray(inputs["bn2_g"], f32)
    bn2_b = np.asarray(inputs["bn2_b"], f32)
    bn2_m = np.asarray(inputs["bn2_m"], f32)
    bn2_v = np.asarray(inputs["bn2_v"], f32)

    packed = cfg["packed"]
    HB = 32 if packed else H
    s1 = bn1_g / np.sqrt(bn1_v + EPS)
    t1 = s1 * (dw_b - bn1_m) + bn1_b

    def _pp(v):  # [C] -> [DWP, G] per-partition vector, zero-padded blocks
        a = np.zeros((G, 4, HB), f32)
        a[:, :, :H] = v.reshape(G, 4)[:, :, None]
        return np.ascontiguousarray(a.reshape(G, 4 * HB).T)

    bn1s = _pp(s1)
    bn1b = _pp(t1 + 3.0)

    # Toeplitz blockdiag: toep[g, ci*28+hin, dx, cj*28+hout]
    #   = dw_w[4g+ci, 0, hin-hout+2, dx] if ci==cj and |hin-hout|<=2
    hin = np.arange(H)[:, None]
    hout = np.arange(H)[None, :]
    D = hin - hout
    mask = np.abs(D) <= 2
    dyi = np.clip(D + 2, 0, 4)
    k = dw_w[:, 0]                                                # [C, 5, 5]
    # band[c, hin, hout, dx]
    band = np.where(mask[None, :, :, None], k[:, dyi, :], 0.0)    # [C,28,28,5]
    band_r = band.reshape(G, 4, H, H, KK)           # [g, ci, hin, hout, dx]
    if packed:
        # [G, 4, 32(hin), KK, 32(hout)] zero-padded per-channel blocks
        toep = np.zeros((G, 4, 32, KK, 32), f32)
        toep[:, :, :H, :, :H] = band_r.transpose(0, 1, 2, 4, 3)
    elif cfg["toep_compact"]:
        # [G, 4, hin, KK, hout] dense diagonal blocks only
        toep = np.ascontiguousarray(band_r.transpose(0, 1, 2, 4, 3))
    else:
        toep = np.zeros((G, 4, H, KK, 4, H), f32)
        for ci in range(4):
            # [g, hin, dx, hout]
            toep[:, ci, :, :, ci, :] = band_r[:, ci].transpose(0, 1, 3, 2)
        toep = toep.reshape(G, 112, KK, 112)
    if cfg["toep_store"] == "float16" and cfg["mm_dt"] != "bfloat16":
        toep = toep.astype(np.float16)
    else:
        toep = toep.astype(mmnp)

    actnp = _NPDT[cfg["act_dt"]]
    pwT = np.ascontiguousarray(pw_w.T)               # [C, Cout]
    pwl = np.zeros((2, 120, 2, 120), f32)
    for kc in range(2):
        for mo in range(2):
            pwl[kc, :, mo, :] = pwT[kc * 120:(kc + 1) * 120,
                                    mo * 120:(mo + 1) * 120]
    pwl = pwl.astype(actnp)

    se1l = np.ascontiguousarray((se_w1.T / HW).reshape(2, 120, R))
    se1b = se_b1.reshape(R, 1).copy()
    se2l = np.ascontiguousarray(se_w2.T.reshape(R, 2, 120))
    se2b3 = np.ascontiguousarray((se_b2 + 3.0).reshape(2, 120).T)
    s2 = bn2_g / np.sqrt(bn2_v + EPS)
    bn2s = np.ascontiguousarray(s2.reshape(2, 120).T)
    bn2sb = np.ascontiguousarray((s2 * pw_b).reshape(2, 120).T)
    bn2t = np.ascontiguousarray((bn2_b - bn2_m * s2).reshape(2, 120).T)

    shared = {
        "toep": toep, "bn1s": bn1s, "bn1b": bn1b, "pwl": pwl,
        "se1l": se1l.astype(f32), "se1b": se1b, "se2l": se2l.astype(f32),
        "se2b3": se2b3, "bn2s": bn2s, "bn2sb": bn2sb, "bn2t": bn2t,
        "zeros": np.zeros((128, 640), mmnp),
    }
    in_maps = []
    for i in range(N_CORES):
        m = dict(shared)
        m["x"] = np.ascontiguousarray(x[i * NB:(i + 1) * NB])
        in_maps.append(m)
    return in_maps


def kernel(**inputs):
    nc = build_nc()
    in_maps = prep_inputs(inputs)
    res = run_bass_kernel_spmd(nc, in_maps, list(range(N_CORES)))
    out = np.concatenate([res.results[i]["y"] for i in range(N_CORES)], axis=0)
    return out.astype(np.float32)



# revision 63
# speedup vs baseline: 2.1033x; 2.1033x over previous
"""Trainium2 Bass kernel for nn_DepthWiseSepConv (depthwise 5x5 + BN+hardswish
+ pointwise 1x1 + squeeze-excite gating + BN), data-parallel over batch on
8 NeuronCores.

v2 design (all-fp16 matmul operands, DMA-lean):
  - x pre-padded on host to [112=(c4,h), 60g, 8b, 36w] fp16 -> 4 chunked DMAs,
    no on-device zero fill, large contiguous runs.
  - toep: full block-diagonal Toeplitz [112, 60g, 5dx, 112] fp16 built on
    host -> 6 chunked DMAs.
  - DW: per group, 5 matmuls (dx taps) accumulate psum [112, 8, 28].
    Epilogue: Act relu(scale,bias) -> fp16, DVE min / stt (2x fp16 mode),
    Pool w-reduce into sacc (SE mean partials), Act copy of T1 transpose.
  - SE: sel4 matmul folds (c4,h)->c4, then 60 tiny matmuls (ap=8)
    accumulate ps1 = w1 @ s; runs on PE tail of phase A, no big reduce.
  - T2 -> channel-major PWrhs fp16; copies split across Act/DVE/Pool.
  - PW: [120x120] fp16 matmuls; epilogue folded into ONE activation with
    scale=bn2s*gate, bias=bn2sb*gate+bn2t (per (mo,b) vectors from SE).
  - y written fp16, cast to f32 on host.
"""

import sys

sys.path.insert(0, "/opt/trn_rl_repo")

import numpy as np
import ml_dtypes

import concourse.bass as bass
import concourse.mybir as mybir
import concourse.tile as tile
from concourse import bacc
from concourse.bass_utils import run_bass_kernel_spmd
from concourse.masks import make_identity

# ---------------------------------------------------------------- constants
N_CORES = 8
B, C, H, W = 64, 240, 28, 28
NB = B // N_CORES          # images per core
KK = 5                      # depthwise kernel size
G = C // 4                  # channel groups of 4 -> 60
R = 60                      # SE reduction dim
Cout = 240
HW = H * W                  # 784
EPS = 1e-5
WP = 36                     # padded W in x DRAM/SBUF (w in [-2, 34))
PIX = NB * HW               # 6272 pixels per core
P112 = 4 * H                # (c4, h) partition count

# g-split chunking for pipelined input DMAs (pair-aligned: all even)
TOEP_CHUNKS = [4, 8, 12, 12, 12, 12]
X_CHUNKS = [4, 12, 14, 30]

CFG = {
    # dtype of DW matmul operands
    "dw_dt": "float16",
    # dtype of PW matmul operands / activations
    "act_dt": "float16",
    # engine split for T2 copies: counts per (dve, act) cycle; Pool cannot
    # read PSUM on real HW
    "t2_split": (2, 1, 0),
    # PW epilogue engine cycle: a=Act, d=DVE (Pool cannot read PSUM)
    "pw_split": "adadadad",
}

_DT = {
    "float32": mybir.dt.float32,
    "float16": mybir.dt.float16,
    "bfloat16": mybir.dt.bfloat16,
    "float8e4": mybir.dt.float8e4,
    "float8e3": mybir.dt.float8e3,
}
_NPDT = {
    "float32": np.float32,
    "float16": np.float16,
    "bfloat16": ml_dtypes.bfloat16,
    "float8e4": ml_dtypes.float8_e4m3,
    "float8e3": ml_dtypes.float8_e3m4,
}

AL = mybir.AluOpType
AF = mybir.ActivationFunctionType


def _chunk_offsets(chunks):
    offs = [0]
    for c in chunks:
        offs.append(offs[-1] + c)
    return offs


# ---------------------------------------------------------------- builder
_BUILD_CACHE = {}


def build_nc(cfg_key=None):
    cfg = dict(CFG)
    if cfg_key is not None:
        cfg.update(cfg_key)
    key = tuple(sorted((k, str(v)) for k, v in cfg.items()))
    if key in _BUILD_CACHE:
        return _BUILD_CACHE[key]

    dw_dt = _DT[cfg["dw_dt"]]
    act_dt = _DT[cfg["act_dt"]]
    f32 = mybir.dt.float32
    f16 = mybir.dt.float16

    nc = bacc.Bacc("TRN2", target_bir_lowering=False, debug=False,
                   num_devices=N_CORES)

    # K=113: row 112 is a constant-ones row in x / the folded BN1 bias row in
    # toep (only in the dx=0 slice), so psum = BN1-affine(conv) directly.
    x_p = nc.declare_dram_parameter("x16", [P112 + 1, G, NB, WP], dw_dt,
                                    isOutput=False)
    toep_p = nc.declare_dram_parameter("toep", [P112 + 1, G, KK, P112], dw_dt,
                                       isOutput=False)
    pwl_p = nc.declare_dram_parameter("pwl", [120, 2, 2, 120], act_dt,
                                      isOutput=False)
    sel4_p = nc.declare_dram_parameter("sel4", [P112, 4], f16, isOutput=False)
    w1g_p = nc.declare_dram_parameter("w1g", [4, G, R], f16, isOutput=False)
    se2l_p = nc.declare_dram_parameter("se2l", [R, 2, 120], f16,
                                       isOutput=False)
    # f32 constant pack: col 0 se1b(rows<60), 1:3 se2b3, 3:5 bn2s,
    # 5:7 bn2sb, 7:9 bn2t (rows<120)
    cpack_p = nc.declare_dram_parameter("cpack", [128, 9], f32,
                                        isOutput=False)
    y_p = nc.declare_dram_parameter("y", [NB, 2, 120, HW], f16, isOutput=True)

    t_offs = _chunk_offsets(TOEP_CHUNKS)
    x_offs = _chunk_offsets(X_CHUNKS)

    with tile.TileContext(nc) as tc:
        cst = tc.alloc_tile_pool(name="cst", bufs=1)
        pers = tc.alloc_tile_pool(name="pers", bufs=1)

        # ---- constants in SBUF (DMAs issued after the first input chunks)
        cpack_sb = cst.tile([128, 9], f32)
        pwl_sb = cst.tile([120, 2, 2, 120], act_dt)
        sel4_sb = cst.tile([P112, 4], f16)
        w1g_sb = cst.tile([4, G, R], f16)
        se2l_sb = cst.tile([R, 2, 120], f16)

        se1b = cpack_sb[0:R, 0:1]
        se2b3 = cpack_sb[0:120, 1:3]
        bn2s = cpack_sb[0:120, 3:5]
        bn2sb = cpack_sb[0:120, 5:7]
        bn2t = cpack_sb[0:120, 7:9]

        ident = cst.tile([128, 128], act_dt)
        make_identity(nc, ident[:])

        # ---- persistent buffers (chunked input tiles for pipelined DMA)
        toep_tiles = []
        for i, n in enumerate(TOEP_CHUNKS):
            t = pers.tile([P112 + 1, n, KK, P112], dw_dt, name=f"toep_{i}")
            toep_tiles.append(t)
        x_tiles = []
        for i, n in enumerate(X_CHUNKS):
            t = pers.tile([P112 + 1, n, NB, WP], dw_dt, name=f"x_{i}")
            x_tiles.append(t)

        # interleaved prefetch order: early groups first, then constants
        def dma_t(i):
            nc.sync.dma_start(toep_tiles[i][:],
                              toep_p[:, t_offs[i]:t_offs[i + 1]])

        def dma_x(i):
            nc.sync.dma_start(x_tiles[i][:],
                              x_p[:, x_offs[i]:x_offs[i + 1]])

        # input chunks lead (they gate the DW matmul stream; DMA_ENGINES is
        # ~95% busy during phase A so consts slot in after the second chunk)
        dma_t(0); dma_x(0); dma_t(1); dma_x(1)
        nc.sync.dma_start(cpack_sb[:], cpack_p[:])
        nc.sync.dma_start(sel4_sb[:], sel4_p[:])
        nc.sync.dma_start(w1g_sb[:], w1g_p[:])
        dma_t(2); dma_x(2)
        nc.sync.dma_start(pwl_sb[:], pwl_p[:])
        nc.sync.dma_start(se2l_sb[:], se2l_p[:])
        dma_t(3); dma_x(3)
        dma_t(4); dma_t(5)

        def g_tile(tiles, offs, g):
            for i in range(len(offs) - 1):
                if g < offs[i + 1]:
                    return tiles[i], g - offs[i]
            raise AssertionError

        # ActT[ch]: [(b4,w28)=112, q2, gl30, c4, h28] fp16
        ActT = [pers.tile([P112, 2, 30, 4, H], act_dt, name=f"actt_{ch}")
                for ch in range(2)]
        # PWrhs[ch]: [c=120, b8, h28, w28]
        PWrhs = [pers.tile([120, NB, H, W], act_dt, name=f"pwrhs_{ch}")
                 for ch in range(2)]
        sacc = pers.tile([P112, G, NB], f16, name="sacc")
        s4_sb = pers.tile([4, G, NB], f16, name="s4")
        h1_sb = pers.tile([R, NB], f16, name="h1")
        gs_sb = pers.tile([120, 2, NB], f32, name="gs")
        gb_sb = pers.tile([120, 2, NB], f32, name="gb")

        # ================= Phase A: depthwise + BN1 + hardswish + T1 + SE prep
        # PSUM scope 1: dwps(4) + t1ps(2) + seps(2) = 8 banks. The SE squeeze
        # shares this scope so there is a single pool-transition barrier in
        # the whole kernel (between scope 1 and scope 2).
        with tc.tile_pool(name="pa", bufs=5) as pa, \
             tc.tile_pool(name="dwps", bufs=4, space="PSUM") as dwps, \
             tc.tile_pool(name="seps", bufs=1, space="PSUM") as seps, \
             tc.tile_pool(name="t1ps", bufs=3, space="PSUM") as t1ps:
            # Pair-processing: groups (2i, 2i+1) share one psum tile and one
            # epilogue instruction each (relu / min / hswish-mult / w-reduce /
            # T1 copy), halving per-instruction overheads. BN1 is folded into
            # the matmul (scale into toep rows, bias via the K=113 ones-row),
            # so the relu needs no per-group scale/bias.
            # Software pipeline: T1 transposes for pair i are emitted after
            # the DW matmuls of pair i+LAG so PE never waits on the epilogue.
            LAG = 2
            NPAIR = G // 2
            act_tiles = {}

            # PE warm-up: ~3us of dummy matmuls on the identity while the
            # first input chunks are still in flight. The cost model runs the
            # PE at half speed until it has been continuously busy for 3us;
            # without this the first ~80 real matmuls pay double.
            psw = dwps.tile([P112, 2, NB, W], f32, tag="dw")
            wrhs = ident[:P112, :P112].rearrange("p (b w) -> p b w", b=4)
            for _ in range(24):
                nc.tensor.matmul(psw[:, 0, 0:4, :], ident[:P112, :P112],
                                 wrhs, start=True, stop=True)

            def emit_dw(i):
                g0 = 2 * i
                ps = dwps.tile([P112, 2, NB, W], f32, tag="dw")
                for j in range(2):
                    g = g0 + j
                    tt, tg = g_tile(toep_tiles, t_offs, g)
                    xt, xg = g_tile(x_tiles, x_offs, g)
                    for dx in range(KK):
                        nc.tensor.matmul(
                            ps[:, j], tt[:, tg, dx, :],
                            xt[:, xg, :, dx:dx + W],
                            start=(dx == 0), stop=(dx == KK - 1))
                # a = relu(ps)  (Act, fp16 out; BN1 affine already in psum)
                a_g = pa.tile([P112, 2, NB, W], act_dt, tag="a_g")
                nc.scalar.activation(a_g[:], ps[:], AF.Relu)
                # m = min(a/6, 1)   (DVE ts: all-SBUF fp16 -> 4x mode)
                m_g = pa.tile([P112, 2, NB, W], act_dt, tag="m_g")
                nc.vector.tensor_scalar(m_g[:], a_g[:], 1.0 / 6.0, 1.0,
                                        AL.mult, AL.min)
                # n = a - 3   (Pool; SBUF-only so GPSIMD is legal here)
                n_g = pa.tile([P112, 2, NB, W], act_dt, tag="n_g")
                nc.gpsimd.tensor_scalar(n_g[:], a_g[:], -3.0, None, AL.add)
                # act = n * m  (DVE tt: fp16 -> 2x mode)
                act_g = pa.tile([P112, 2, NB, W], act_dt, tag="act_g")
                nc.vector.tensor_tensor(act_g[:], n_g[:], m_g[:], AL.mult)
                act_tiles[i] = act_g

            def emit_t1(i):
                act_g = act_tiles.pop(i)
                tp = t1ps.tile([P112, 2, 2, P112], act_dt, tag="t1")
                for j in range(2):
                    for q in range(2):
                        nc.tensor.transpose(tp[:, j, q, :],
                                            act_g[:, j, 4 * q:4 * q + 4, :],
                                            ident[:P112, :P112])
                g0 = 2 * i
                ch, gl = (0, g0) if g0 < 30 else (1, g0 - 30)
                # PSUM source -> GPSIMD illegal on HW; Act does this copy
                src = tp[:].rearrange("p j q (c e) -> p q j c e", c=4)
                dst = ActT[ch][:, :, gl:gl + 2, :, :]
                nc.scalar.copy(dst, src)
                # SE mean partial: sum over w -> [112, 2, 8]. Emitted with a
                # lag so this non-critical 527ns DVE op never sits ahead of
                # the chain-critical min/tt of newer pairs in DVE's in-order
                # queue.
                with nc.allow_low_precision(
                        reason="fp16 w-sum of 28 bounded terms; SE tolerant"):
                    nc.vector.tensor_reduce(
                        sacc[:, g0:g0 + 2, :],
                        act_g[:].rearrange("p j b w -> p (j b) w"),
                        mybir.AxisListType.X, AL.add)

            for i in range(NPAIR):
                emit_dw(i)
                if i >= LAG:
                    emit_t1(i - LAG)
            for i in range(NPAIR - LAG, NPAIR):
                emit_t1(i)

            # SE squeeze: fold (c4,h)->c4, then ps1 += w1g[g].T @ s4[g]
            # ps1 and s4ps share one 2KB psum bank (disjoint column ranges)
            sps = seps.tile([64, 488], f32, name="sps")
            ps1 = sps[0:R, 480:488]
            s4ps = sps[0:4, 0:480].rearrange("p (g b) -> p g b", g=G)
            nc.tensor.matmul(s4ps, sel4_sb[:], sacc[:],
                             start=True, stop=True)
            nc.vector.tensor_copy(s4_sb[:], s4ps)
            for g in range(G):
                nc.tensor.matmul(ps1, w1g_sb[:, g, :], s4_sb[:, g, :],
                                 start=(g == 0), stop=(g == G - 1))
            nc.scalar.activation(h1_sb[:], ps1, AF.Relu, bias=se1b)

        # ================= Phases C+B+D: PSUM scope 2
        # t2ps(4) + seps2(1) + pdps(3) = 8 banks
        with tc.tile_pool(name="se", bufs=1) as sep, \
             tc.tile_pool(name="seps2", bufs=1, space="PSUM") as seps2:
            for mo in range(2):
                ps2 = seps2.tile([120, NB], f32, tag="se2")
                nc.tensor.matmul(ps2[:], se2l_sb[:, mo, :], h1_sb[:],
                                 start=True, stop=True)
                a2 = sep.tile([120, NB], f32, name=f"a2_{mo}")
                nc.scalar.activation(a2[:], ps2[:], AF.Relu,
                                     bias=se2b3[:, mo:mo + 1])
                m2 = sep.tile([120, NB], f32, name=f"m2_{mo}")
                nc.vector.tensor_scalar(m2[:], a2[:], 1.0 / 6.0, 1.0,
                                        AL.mult, AL.min)
                g2 = sep.tile([120, NB], f32, name=f"g2_{mo}")
                nc.vector.scalar_tensor_tensor(g2[:], a2[:], 3.0, m2[:],
                                               AL.subtract, AL.mult)
                # gs = g2 * bn2s ; gb = g2 * bn2sb + bn2t
                nc.vector.tensor_scalar(gs_sb[:, mo, :], g2[:],
                                        bn2s[:, mo:mo + 1], None, AL.mult)
                nc.vector.tensor_scalar(gb_sb[:, mo, :], g2[:],
                                        bn2sb[:, mo:mo + 1],
                                        bn2t[:, mo:mo + 1],
                                        AL.mult, AL.add)

            # ============= Phase B: T2 -> channel-major PWrhs
            t2ps = tc.alloc_tile_pool(name="t2ps", bufs=6, space="PSUM")
            d, a, p = cfg["t2_split"]
            cyc = ["dve"] * d + ["act"] * a + ["pool"] * p
            for ch in range(2):
                for q in range(2):
                    for ih, h0 in enumerate(range(0, H, 4)):
                        tp2 = t2ps.tile([120, 4, P112], act_dt, tag="t2")
                        for hi in range(4):
                            nc.tensor.transpose(
                                tp2[:, hi, :],
                                ActT[ch][:, q, :, :, h0 + hi],
                                ident[:P112, :P112])
                        src = tp2[:].rearrange(
                            "p hh (b w) -> p b hh w", b=4)
                        dst = PWrhs[ch][:, 4 * q:4 * q + 4, h0:h0 + 4, :]
                        eng = cyc[(ch * 14 + q * 7 + ih) % len(cyc)]
                        if eng == "dve":
                            nc.vector.tensor_copy(dst, src)
                        elif eng == "act":
                            nc.scalar.copy(dst, src)
                        else:
                            nc.gpsimd.tensor_copy(dst, src)

            # ========= Phase D: pointwise conv + fused epilogue + output
            # b-major loop so each image's 4 tiles finish together -> one
            # y DMA per image (8 DMAs instead of 32).
            NT = 392  # half an image
            t2ps.release()
            pd = tc.alloc_tile_pool(name="pd", bufs=3)
            pdps = tc.alloc_tile_pool(name="pdps", bufs=4, space="PSUM")
            ti = 0
            for b in range(NB):
                o_b = pd.tile([120, 2, HW], f16, tag="o_b")
                for nt in range(2):
                    for mo in range(2):
                        ps = pdps.tile([120, NT], f32, tag="pw")
                        for kc in range(2):
                            nc.tensor.matmul(
                                ps[:],
                                pwl_sb[:, kc, mo, :],
                                PWrhs[kc][:].rearrange(
                                    "p b hh w -> p (b hh w)")[
                                    :, b * HW + nt * NT:
                                    b * HW + (nt + 1) * NT],
                                start=(kc == 0), stop=(kc == 1))
                        o_t = o_b[:, mo, nt * NT:(nt + 1) * NT]
                        eng = cfg["pw_split"][ti % len(cfg["pw_split"])]
                        if eng == "d":
                            nc.vector.tensor_scalar(
                                o_t, ps[:],
                                gs_sb[:, mo, b:b + 1],
                                gb_sb[:, mo, b:b + 1],
                                AL.mult, AL.add)
                        elif eng == "p":
                            nc.gpsimd.tensor_scalar(
                                o_t, ps[:],
                                gs_sb[:, mo, b:b + 1],
                                gb_sb[:, mo, b:b + 1],
                                AL.mult, AL.add)
                        else:
                            nc.scalar.activation(
                                o_t, ps[:], AF.Identity,
                                bias=gb_sb[:, mo, b:b + 1],
                                scale=gs_sb[:, mo, b:b + 1])
                        ti += 1
                    # one DMA per half-image as soon as both mo tiles land
                    nc.sync.dma_start(
                        y_p[b, :, :, nt * NT:(nt + 1) * NT].rearrange(
                            "mo c f -> c mo f"),
                        o_b[:, :, nt * NT:(nt + 1) * NT])

            pdps.release()
            pd.release()

        pers.release()
        cst.release()

    nc.compile()
    _BUILD_CACHE[key] = nc
    return nc


# ---------------------------------------------------------------- host prep
def prep_inputs(inputs, cfg_key=None):
    cfg = dict(CFG)
    if cfg_key is not None:
        cfg.update(cfg_key)
    dwnp = _NPDT[cfg["dw_dt"]]
    actnp = _NPDT[cfg["act_dt"]]
    f32 = np.float32

    x = np.asarray(inputs["x"], f32)
    dw_w = np.asarray(inputs["dw_w"], f32)      # [C,1,5,5]
    dw_b = np.asarray(inputs["dw_b"], f32)
    bn1_g = np.asarray(inputs["bn1_g"], f32)
    bn1_b = np.asarray(inputs["bn1_b"], f32)
    bn1_m = np.asarray(inputs["bn1_m"], f32)
    bn1_v = np.asarray(inputs["bn1_v"], f32)
    pw_w = np.asarray(inputs["pw_w"], f32)      # [Cout, C]
    pw_b = np.asarray(inputs["pw_b"], f32)
    se_w1 = np.asarray(inputs["se_w1"], f32)    # [R, C]
    se_b1 = np.asarray(inputs["se_b1"], f32)
    se_w2 = np.asarray(inputs["se_w2"], f32)    # [Cout, R]
    se_b2 = np.asarray(inputs["se_b2"], f32)
    bn2_g = np.asarray(inputs["bn2_g"], f32)
    bn2_b = np.asarray(inputs["bn2_b"], f32)
    bn2_m = np.asarray(inputs["bn2_m"], f32)
    bn2_v = np.asarray(inputs["bn2_v"], f32)

    s1 = bn1_g / np.sqrt(bn1_v + EPS)
    t1 = s1 * (dw_b - bn1_m) + bn1_b

    # Toeplitz full block-diag [113=(c4,hin)+bias, G, KK(dx), 112=(c4,hout)]
    # with BN1 scale folded into the weights and BN1 bias (+3 for the
    # hardswish shift) in row 112 of the dx=0 slice.
    hin = np.arange(H)[:, None]
    hout = np.arange(H)[None, :]
    D = hin - hout
    mask = np.abs(D) <= 2
    dyi = np.clip(D + 2, 0, 4)
    k = dw_w[:, 0] * s1[:, None, None]                            # [C, 5, 5]
    band = np.where(mask[None, :, :, None], k[:, dyi, :], 0.0)    # [C,28,28,5]
    band_r = band.reshape(G, 4, H, H, KK)           # [g, ci, hin, hout, dx]
    toep = np.zeros((P112 + 1, G, KK, 4, H), f32)
    toep_b = toep[:P112].reshape(4, H, G, KK, 4, H)
    for ci in range(4):
        # [hin, g, dx, hout] for channel slot ci
        toep_b[ci, :, :, :, ci, :] = band_r[:, ci].transpose(1, 0, 3, 2)
    # bias row: (t1+3)[4g+c4] for every output column (c4, h), dx=0 only
    toep[P112, :, 0, :, :] = (t1 + 3.0).reshape(G, 4)[:, :, None]
    toep = toep.reshape(P112 + 1, G, KK, P112).astype(dwnp)

    # x16: [112=(c4,h), G, NB(per core), WP] fp16 pre-padded
    # (per-core slices below)

    # pwl [120, 2kc, 2mo, 120]
    pwT = np.ascontiguousarray(pw_w.T)               # [C, Cout]
    pwl = np.zeros((120, 2, 2, 120), f32)
    for kc in range(2):
        for mo in range(2):
            pwl[:, kc, mo, :] = pwT[kc * 120:(kc + 1) * 120,
                                    mo * 120:(mo + 1) * 120]
    pwl = pwl.astype(actnp)

    # sel4: [112=(c4,h), 4] one-hot on c4
    sel4 = np.zeros((4, H, 4), np.float16)
    for c4 in range(4):
        sel4[c4, :, c4] = 1.0
    sel4 = sel4.reshape(P112, 4)

    # w1g [4, G, R] = se_w1[r, 4g+c4] / HW
    w1g = (se_w1.T.reshape(G, 4, R).transpose(1, 0, 2) / HW
           ).astype(np.float16)
    se2l = np.ascontiguousarray(
        se_w2.T.reshape(R, 2, 120)).astype(np.float16)

    s2 = bn2_g / np.sqrt(bn2_v + EPS)
    cpack = np.zeros((128, 9), f32)
    cpack[0:R, 0] = se_b1
    cpack[0:120, 1:3] = (se_b2 + 3.0).reshape(2, 120).T
    cpack[0:120, 3:5] = s2.reshape(2, 120).T
    cpack[0:120, 5:7] = (s2 * pw_b).reshape(2, 120).T
    cpack[0:120, 7:9] = (bn2_b - bn2_m * s2).reshape(2, 120).T

    shared = {
        "toep": toep, "pwl": pwl, "sel4": sel4, "w1g": w1g, "se2l": se2l,
        "cpack": cpack,
    }
    in_maps = []
    for i in range(N_CORES):
        m = dict(shared)
        xc = x[i * NB:(i + 1) * NB]                  # [NB, C, H, W]
        x16 = np.zeros((P112 + 1, G, NB, WP), dwnp)
        # [b, g, c4, h, w] -> [c4, h, g, b, w]
        x16[:P112].reshape(4, H, G, NB, WP)[:, :, :, :, 2:2 + W] = (
            xc.reshape(NB, G, 4, H, W).transpose(2, 3, 1, 0, 4).astype(dwnp))
        x16[P112] = 1.0                              # bias ones-row
        m["x16"] = x16
        in_maps.append(m)
    return in_maps


def kernel(**inputs):
    nc = build_nc()
    in_maps = prep_inputs(inputs)
    res = run_bass_kernel_spmd(nc, in_maps, list(range(N_CORES)))
    outs = []
    for i in range(N_CORES):
        y = np.asarray(res.results[i]["y"], dtype=np.float32)  # [NB,2,120,HW]
        outs.append(y.reshape(NB, Cout, H, W))
    return np.concatenate(outs, axis=0)
